# revision 1
# baseline (speedup 1.0000x reference)
"""MoE layer (nn_MoELayer_84971632984718) Trainium2 Bass kernel.

Strategy (expert-parallel, top-2 sparse):
  - Host: router matmul [B*S,H]@[H,E] (0.016% of total FLOPs), top-2 +
    softmax gating, aux losses. Tokens are gathered per expert.
  - Device (8 cores, SPMD): core e runs expert e's FFN over its gathered
    tokens, padded to shared capacity C:
        hT = gelu(W1^T x^T + b1)   via  matmul(lhsT=W1[H,I], rhs=xT[H,C])
        yT = W2^T hT + b2          via  matmul(lhsT=W2[I,H], rhs=hT[I,C])
    Everything stays transposed (tokens on the free dim) so no on-device
    transposes are needed; fp16 operands, fp32 PSUM accumulation.
  - Host: scatter-add of gate-weighted expert outputs (the "weighted
    combine" all-reduce equivalent, 0.004% of FLOPs).

Only the top-2 experts per token have nonzero gate weight, so this is
numerically the same final_output as the reference's dense all-expert
compute at 1/4 of the FLOPs.
"""
import sys

if "/opt/trn_rl_repo" not in sys.path:
    sys.path.insert(0, "/opt/trn_rl_repo")

import numpy as np

import concourse.mybir as mybir
import concourse.tile as tile
from concourse import bacc
from concourse.bass_utils import run_bass_kernel_spmd

B, S, H, I, E = 4, 2048, 768, 3072, 8
TOP_K = 2
ALPHA_BALANCE = 0.01
ENERGY_ALPHA = 0.0

N_CORES = 8
P = 128
HT = H // P   # 6  k-tiles (stage 1) / m-tiles (stage 2)
IT = I // P   # 24 m-tiles (stage 1) / k-tiles (stage 2)
NB = 512      # psum bank free-dim (fp32)
C_STEP = 512  # capacity granularity

MM_DT = mybir.dt.float16
NP_DT = np.float16


# ---------------------------------------------------------------- device ---

def _build(C: int):
    """Build + compile the SPMD FFN kernel for token capacity C."""
    nc = bacc.Bacc("TRN2", target_bir_lowering=False, debug=False)
    xT_d = nc.dram_tensor("xT", [H, C], MM_DT, kind="ExternalInput")
    w1_d = nc.dram_tensor("w1", [H, I], MM_DT, kind="ExternalInput")
    w2_d = nc.dram_tensor("w2", [I, H], MM_DT, kind="ExternalInput")
    b1_d = nc.dram_tensor("b1", [P, IT], mybir.dt.float32, kind="ExternalInput")
    b2_d = nc.dram_tensor("b2", [P, HT], mybir.dt.float32, kind="ExternalInput")
    yT_d = nc.dram_tensor("yT", [H, C], mybir.dt.float32, kind="ExternalOutput")

    n_chunks = (C + NB - 1) // NB
    chunks = [(ci * NB, min(NB, C - ci * NB)) for ci in range(n_chunks)]

    with tile.TileContext(nc) as tc:
        with (
            tc.tile_pool(name="wpool", bufs=1) as wpool,
            tc.tile_pool(name="xpool", bufs=1) as xpool,
            tc.tile_pool(name="hpool", bufs=2) as hpool,
            tc.tile_pool(name="ypool", bufs=3) as ypool,
            tc.tile_pool(name="ps1", bufs=4, space="PSUM") as ps1,
            tc.tile_pool(name="ps2", bufs=4, space="PSUM") as ps2,
        ):
            w1_sb = wpool.tile([P, HT, I], MM_DT)
            w2_sb = wpool.tile([P, IT, H], MM_DT)
            b1_sb = wpool.tile([P, IT], mybir.dt.float32)
            b2_sb = wpool.tile([P, HT], mybir.dt.float32)
            xT_sb = xpool.tile([P, HT, C], MM_DT)

            nc.sync.dma_start(b1_sb[:], b1_d[:])
            nc.sync.dma_start(b2_sb[:], b2_d[:])
            # split big loads so compute can start as soon as its slice lands
            for m in range(IT):
                nc.sync.dma_start(
                    w1_sb[:, :, m * P : (m + 1) * P],
                    w1_d.rearrange("(kt p) i -> p kt i", p=P)[
                        :, :, m * P : (m + 1) * P
                    ],
                )
            for k in range(IT):
                nc.sync.dma_start(
                    w2_sb[:, k, :],
                    w2_d.rearrange("(kt p) h -> p kt h", p=P)[:, k, :],
                )
            nc.sync.dma_start(
                xT_sb[:], xT_d.rearrange("(kt p) c -> p kt c", p=P)
            )

            for c0, cn in chunks:
                hT_sb = hpool.tile([P, IT, NB], MM_DT, name="hT")
                # ---- stage 1: hT[m] = gelu(W1[:,m].T @ xT + b1[m]) ----
                for m in range(IT):
                    psum = ps1.tile([P, NB], mybir.dt.float32, name="ps1t")
                    for k in range(HT):
                        nc.tensor.matmul(
                            psum[:, :cn],
                            w1_sb[:, k, m * P : (m + 1) * P],
                            xT_sb[:, k, c0 : c0 + cn],
                            start=(k == 0),
                            stop=(k == HT - 1),
                        )
                    nc.scalar.activation(
                        hT_sb[:, m, :cn],
                        psum[:, :cn],
                        mybir.ActivationFunctionType.Gelu,
                        bias=b1_sb[:, m : m + 1],
                    )
                # ---- stage 2: yT[m] = W2[:,m].T @ hT + b2[m] ----
                for m in range(HT):
                    psum = ps2.tile([P, NB], mybir.dt.float32, name="ps2t")
                    for k in range(IT):
                        nc.tensor.matmul(
                            psum[:, :cn],
                            w2_sb[:, k, m * P : (m + 1) * P],
                            hT_sb[:, k, :cn],
                            start=(k == 0),
                            stop=(k == IT - 1),
                        )
                    y_sb = ypool.tile([P, NB], mybir.dt.float32, name="y")
                    nc.vector.tensor_scalar_add(
                        y_sb[:, :cn], psum[:, :cn], b2_sb[:, m : m + 1]
                    )
                    nc.sync.dma_start(
                        yT_d.rearrange("(mt p) c -> p mt c", p=P)[
                            :, m, c0 : c0 + cn
                        ],
                        y_sb[:, :cn],
                    )
    nc.compile()
    return nc


_NC_CACHE: dict = {}


def _get_nc(C: int):
    if C not in _NC_CACHE:
        _NC_CACHE[C] = _build(C)
    return _NC_CACHE[C]


# ------------------------------------------------------------------ host ---

def _route(hidden_flat, router_w, router_b):
    """Replicate the reference routing exactly (top-2, softmax, dense gate)."""
    logits = hidden_flat @ router_w + router_b  # [T, E] fp32
    # jax.lax.top_k: descending values, ties -> lower index first
    order = np.argsort(-logits, axis=-1, kind="stable")
    top_i = order[:, :TOP_K]  # [T, 2]
    top_v = np.take_along_axis(logits, top_i, axis=-1)
    m = top_v.max(axis=-1, keepdims=True)
    e = np.exp(top_v - m)
    top_w = (e / e.sum(axis=-1, keepdims=True)).astype(np.float32)
    T = logits.shape[0]
    gating = np.zeros((T, E), np.float32)
    np.put_along_axis(gating, top_i, top_w, axis=-1)
    return gating, top_i, top_w


def kernel(hidden_states, router_w, router_b, w1, b1, w2, b2):
    hidden_states = np.asarray(hidden_states)
    router_w = np.asarray(router_w)
    router_b = np.asarray(router_b)
    w1 = np.asarray(w1)
    b1 = np.asarray(b1)
    w2 = np.asarray(w2)
    b2 = np.asarray(b2)

    T = B * S
    x = hidden_states.reshape(T, H)
    gating, top_i, top_w = _route(x, router_w, router_b)

    # --- aux losses (match reference formulas) ---
    importance = gating.sum(axis=0)  # [E]
    mean_imp = importance.mean()
    load_balancing_loss = ALPHA_BALANCE * (
        np.var(importance, ddof=1) / (mean_imp**2 + 1e-8)
    )
    probs_mean = gating.mean(axis=0)
    energy_loss = ENERGY_ALPHA * np.sum(probs_mean**2)
    aux_loss = np.float32(load_balancing_loss + energy_loss)

    # --- per-expert token gather ---
    tok_idx = [np.where(gating[:, e] > 0)[0] for e in range(E)]
    counts = [len(ix) for ix in tok_idx]
    C = max(C_STEP, ((max(counts) + C_STEP - 1) // C_STEP) * C_STEP)

    nc = _get_nc(C)

    in_maps = []
    for e in range(E):
        ix = tok_idx[e]
        xT = np.zeros((H, C), NP_DT)
        xT[:, : counts[e]] = x[ix].T.astype(NP_DT)
        in_maps.append(
            {
                "xT": xT,
                "w1": w1[e].astype(NP_DT),
                "w2": w2[e].astype(NP_DT),
                "b1": np.ascontiguousarray(
                    b1[e].reshape(IT, P).T.astype(np.float32)
                ),
                "b2": np.ascontiguousarray(
                    b2[e].reshape(HT, P).T.astype(np.float32)
                ),
            }
        )

    res = run_bass_kernel_spmd(nc, in_maps, list(range(N_CORES)))

    # --- weighted combine (host scatter-add; gate==0 experts contribute 0) ---
    final = np.zeros((T, H), np.float32)
    for e in range(E):
        ix = tok_idx[e]
        yT = res.results[e]["yT"]  # [H, C] fp32
        final[ix] += gating[ix, e : e + 1] * yT[:, : counts[e]].T

    return (
        final.reshape(B, S, H),
        aux_loss,
        gating.reshape(B, S, E),
    )


# revision 5
# speedup vs baseline: 68.1527x; 68.1527x over previous
"""MoE layer (nn_MoELayer_84971632984718) Trainium2 Bass kernel.

Strategy (expert-parallel, top-2 sparse):
  - Host: router matmul [B*S,H]@[H,E] (0.016% of total FLOPs), top-2 +
    softmax gating, aux losses. Tokens are gathered per expert.
  - Device (8 cores, SPMD): core e runs expert e's FFN over its gathered
    tokens, padded to shared capacity C:
        hT = gelu(W1^T x^T + b1)   via  matmul(lhsT=W1[H,I], rhs=xT[H,C])
        yT = W2^T hT + b2          via  matmul(lhsT=W2[I,H], rhs=hT[I,C])
    Everything stays transposed (tokens on the free dim) so no on-device
    transposes are needed; fp16 operands, fp32 PSUM accumulation.
  - Host: scatter-add of gate-weighted expert outputs (the "weighted
    combine" all-reduce equivalent, 0.004% of FLOPs).

Only the top-2 experts per token have nonzero gate weight, so this is
numerically the same final_output as the reference's dense all-expert
compute at 1/4 of the FLOPs.
"""
import sys

if "/opt/trn_rl_repo" not in sys.path:
    sys.path.insert(0, "/opt/trn_rl_repo")

import numpy as np

import concourse.mybir as mybir
import concourse.tile as tile
from concourse import bacc
from concourse.bass_utils import run_bass_kernel_spmd

B, S, H, I, E = 4, 2048, 768, 3072, 8
TOP_K = 2
ALPHA_BALANCE = 0.01
ENERGY_ALPHA = 0.0

N_CORES = 8
P = 128
HT = H // P   # 6  k-tiles (stage 1) / m-tiles (stage 2)
IT = I // P   # 24 m-tiles (stage 1) / k-tiles (stage 2)
NB = 512      # psum bank free-dim (fp32)
C_STEP = 128  # capacity granularity

MM_DT = mybir.dt.float16
NP_DT = np.float16


# ---------------------------------------------------------------- device ---

def _build(C: int, reps: int = 1):
    """Build + compile the SPMD FFN kernel for token capacity C.

    reps>1 wraps the compute body in a hardware loop (timing probes only)."""
    nc = bacc.Bacc("TRN2", target_bir_lowering=False, debug=False)
    xT_d = nc.dram_tensor("xT", [H, C], MM_DT, kind="ExternalInput")
    w1_d = nc.dram_tensor("w1", [H, I], MM_DT, kind="ExternalInput")
    w2_d = nc.dram_tensor("w2", [I, H], MM_DT, kind="ExternalInput")
    b1_d = nc.dram_tensor("b1", [P, IT], mybir.dt.float32, kind="ExternalInput")
    b2_d = nc.dram_tensor("b2", [P, HT], mybir.dt.float32, kind="ExternalInput")
    yT_d = nc.dram_tensor("yT", [H, C], mybir.dt.float32, kind="ExternalOutput")

    n_chunks = (C + NB - 1) // NB
    chunks = [(ci * NB, min(NB, C - ci * NB)) for ci in range(n_chunks)]

    with tile.TileContext(nc) as tc:
        with (
            tc.tile_pool(name="wpool", bufs=1) as wpool,
            tc.tile_pool(name="xpool", bufs=1) as xpool,
            tc.tile_pool(name="hpool", bufs=2) as hpool,
            tc.tile_pool(name="ypool", bufs=3) as ypool,
            tc.tile_pool(name="ps1", bufs=4, space="PSUM") as ps1,
            tc.tile_pool(name="ps2", bufs=4, space="PSUM") as ps2,
        ):
            w1_sb = wpool.tile([P, HT, I], MM_DT)
            w2_sb = wpool.tile([P, IT, H], MM_DT)
            b1_sb = wpool.tile([P, IT], mybir.dt.float32)
            b2_sb = wpool.tile([P, HT], mybir.dt.float32)
            xT_sb = xpool.tile([P, HT, C], MM_DT)

            nc.sync.dma_start(b1_sb[:], b1_d[:])
            nc.sync.dma_start(b2_sb[:], b2_d[:])
            # split big loads so compute can start as soon as its slice lands
            for m in range(IT):
                nc.sync.dma_start(
                    w1_sb[:, :, m * P : (m + 1) * P],
                    w1_d.rearrange("(kt p) i -> p kt i", p=P)[
                        :, :, m * P : (m + 1) * P
                    ],
                )
            for k in range(IT):
                nc.sync.dma_start(
                    w2_sb[:, k, :],
                    w2_d.rearrange("(kt p) h -> p kt h", p=P)[:, k, :],
                )
            for c0, cn in chunks:
                nc.sync.dma_start(
                    xT_sb[:, :, c0 : c0 + cn],
                    xT_d.rearrange("(kt p) c -> p kt c", p=P)[:, :, c0 : c0 + cn],
                )

            def body():
                for c0, cn in chunks:
                    hT_sb = hpool.tile([P, IT, NB], MM_DT, name="hT")
                    # ---- stage 1: hT[m] = gelu(W1[:,m].T @ xT + b1[m]) ----
                    for m in range(IT):
                        psum = ps1.tile([P, NB], mybir.dt.float32, name="ps1t")
                        for k in range(HT):
                            nc.tensor.matmul(
                                psum[:, :cn],
                                w1_sb[:, k, m * P : (m + 1) * P],
                                xT_sb[:, k, c0 : c0 + cn],
                                start=(k == 0),
                                stop=(k == HT - 1),
                            )
                        nc.scalar.activation(
                            hT_sb[:, m, :cn],
                            psum[:, :cn],
                            mybir.ActivationFunctionType.Gelu,
                            bias=b1_sb[:, m : m + 1],
                        )
                    # ---- stage 2: yT[m] = W2[:,m].T @ hT + b2[m] ----
                    for m in range(HT):
                        psum = ps2.tile([P, NB], mybir.dt.float32, name="ps2t")
                        for k in range(IT):
                            nc.tensor.matmul(
                                psum[:, :cn],
                                w2_sb[:, k, m * P : (m + 1) * P],
                                hT_sb[:, k, :cn],
                                start=(k == 0),
                                stop=(k == IT - 1),
                            )
                        y_sb = ypool.tile([P, NB], mybir.dt.float32, name="y")
                        nc.vector.tensor_scalar_add(
                            y_sb[:, :cn], psum[:, :cn], b2_sb[:, m : m + 1]
                        )
                        nc.sync.dma_start(
                            yT_d.rearrange("(mt p) c -> p mt c", p=P)[
                                :, m, c0 : c0 + cn
                            ],
                            y_sb[:, :cn],
                        )

            if reps == 1:
                body()
            else:
                with tc.For_i(0, reps, 1):
                    body()
    nc.compile()
    return nc


_NC_CACHE: dict = {}


def _get_nc(C: int):
    if C not in _NC_CACHE:
        _NC_CACHE[C] = _build(C)
    return _NC_CACHE[C]


# ------------------------------------------------------------------ host ---

def _route(hidden_flat, router_w, router_b):
    """Replicate the reference routing exactly (top-2, softmax, dense gate)."""
    logits = hidden_flat @ router_w + router_b  # [T, E] fp32
    # jax.lax.top_k: descending values, ties -> lower index first
    order = np.argsort(-logits, axis=-1, kind="stable")
    top_i = order[:, :TOP_K]  # [T, 2]
    top_v = np.take_along_axis(logits, top_i, axis=-1)
    m = top_v.max(axis=-1, keepdims=True)
    e = np.exp(top_v - m)
    top_w = (e / e.sum(axis=-1, keepdims=True)).astype(np.float32)
    T = logits.shape[0]
    gating = np.zeros((T, E), np.float32)
    np.put_along_axis(gating, top_i, top_w, axis=-1)
    return gating, top_i, top_w


def kernel(hidden_states, router_w, router_b, w1, b1, w2, b2):
    hidden_states = np.asarray(hidden_states)
    router_w = np.asarray(router_w)
    router_b = np.asarray(router_b)
    w1 = np.asarray(w1)
    b1 = np.asarray(b1)
    w2 = np.asarray(w2)
    b2 = np.asarray(b2)

    T = B * S
    x = hidden_states.reshape(T, H)
    gating, top_i, top_w = _route(x, router_w, router_b)

    # --- aux losses (match reference formulas) ---
    importance = gating.sum(axis=0)  # [E]
    mean_imp = importance.mean()
    load_balancing_loss = ALPHA_BALANCE * (
        np.var(importance, ddof=1) / (mean_imp**2 + 1e-8)
    )
    probs_mean = gating.mean(axis=0)
    energy_loss = ENERGY_ALPHA * np.sum(probs_mean**2)
    aux_loss = np.float32(load_balancing_loss + energy_loss)

    # --- per-expert token gather ---
    tok_idx = [np.where(gating[:, e] > 0)[0] for e in range(E)]
    counts = [len(ix) for ix in tok_idx]
    C = max(C_STEP, ((max(counts) + C_STEP - 1) // C_STEP) * C_STEP)

    nc = _get_nc(C)

    in_maps = []
    for e in range(E):
        ix = tok_idx[e]
        xT = np.zeros((H, C), NP_DT)
        xT[:, : counts[e]] = x[ix].T.astype(NP_DT)
        in_maps.append(
            {
                "xT": xT,
                "w1": w1[e].astype(NP_DT),
                "w2": w2[e].astype(NP_DT),
                "b1": np.ascontiguousarray(
                    b1[e].reshape(IT, P).T.astype(np.float32)
                ),
                "b2": np.ascontiguousarray(
                    b2[e].reshape(HT, P).T.astype(np.float32)
                ),
            }
        )

    res = run_bass_kernel_spmd(nc, in_maps, list(range(N_CORES)))

    # --- weighted combine (host scatter-add; gate==0 experts contribute 0) ---
    final = np.zeros((T, H), np.float32)
    for e in range(E):
        ix = tok_idx[e]
        yT = res.results[e]["yT"]  # [H, C] fp32
        final[ix] += gating[ix, e : e + 1] * yT[:, : counts[e]].T

    return (
        final.reshape(B, S, H),
        aux_loss,
        gating.reshape(B, S, E),
    )
